# revision 63
# baseline (speedup 1.0000x reference)
"""Sliding-window multi-head attention (window +-64, S=2048, H=8, hd=64)
for 8 Trainium2 NeuronCores.

Sharding: sequence-parallel. Core c owns queries [c*256, (c+1)*256); it
receives x^T columns for its query range plus a 64-column halo on each side
(zero padded at the sequence edges), computes Q/K/V projections locally
(weights replicated), runs banded softmax-attention for all 8 heads, applies
the output projection, and writes its y^T block. The host reassembles
y = concat_c(yT_c.T) and adds the (input-dependent) constant bias
b_eff = b_o + w_o @ b_v, which is exact because softmax rows sum to 1.

v3 design (all matmul operands bf16, f32 PSUM accumulation):
- per head-PAIR the two heads' score tiles share one [128,512] PSUM bank so
  exp runs as a single wide activation op (no accumulator read);
- the band mask is multiplicative 0/1 applied on DVE via
  scalar_tensor_tensor, whose accum_out yields the masked row sums;
- normalization is folded into the PE transpose: the transpose's moving
  operand is diag(1/sums) instead of the identity;
- attention rows are transposed onto an absolute 3x128-key grid; the AV
  matmuls are split per 128-query half so the never-written corner blocks
  of the grid are never read (no zero-init);
- y^T accumulates across all four o-chunks incrementally as each vals chunk
  lands; output is stored bf16 and upcast on the host.

Self-contained: hardcodes all shapes; no sibling imports.
"""

import numpy as np

import concourse.bass as bass
import concourse.tile as tile
from concourse import bacc, mybir
from concourse.bass_utils import run_bass_kernel_spmd

# problem shapes
S = 2048          # sequence length
E = 512           # embed dim (= d_in)
H = 8             # heads
HD = E // H       # head dim, 64
HWIN = 64         # half window (attend to |q-k| <= 64)
N_CORES = 8
SLOC = S // N_CORES       # queries per core, 256
HALO = SLOC + 2 * HWIN    # local x/k/v span, 384
NT = SLOC // 128          # q tiles per core, 2
KC = HALO // 128          # key chunks per core, 3
SPAN = 256                # keys per q tile: [i-64, i+192)
P = 128

F32 = mybir.dt.float32
F32R = mybir.dt.float32r
BF16 = mybir.dt.bfloat16

# knobs
WARMUP_MMS = 7       # dummy matmuls during the DMA ramp to lift the PE HAM

CHAIN_MODE = "split"

# emission schedule for projections / softmax stages / AV+y
SCHEDULE = [
    ("qk", 0), ("qk", 1), ("sa", 0, 0), ("sa", 0, 1),
    ("sb", 0, 0), ("v", 0), ("sb", 0, 1),
    ("qk", 2), ("sa", 1, 0), ("sa", 1, 1),
    ("sb", 1, 0), ("v", 1), ("sb", 1, 1), ("v", 2), ("pb", 0),
    ("qk", 3), ("sa", 2, 0), ("sa", 2, 1),
    ("sb", 2, 0), ("sb", 2, 1), ("pb", 1),
    ("sa", 3, 0), ("sa", 3, 1), ("pb", 2), ("yo",),
    ("sb", 3, 0), ("sb", 3, 1),
]



def _build_kernel(nc: bass.Bass, reps: int = 1):
    """Emit the SPMD per-core program. All per-core variation comes from the
    input tensors. reps>1 repeats the body inside one NEFF (benchmarking)."""
    act_f = mybir.ActivationFunctionType

    # ---- I/O ----  (host packs everything bf16, SBUF-layout-identical)
    xT = nc.dram_tensor("xT", [P, 4, HALO], BF16, kind="ExternalInput").ap()
    # q+k weights, column-chunked: chunk 0 / chunk 1 / chunks 2-3
    wqk0 = nc.dram_tensor("wqk0", [P, 2, 4, P], BF16,
                          kind="ExternalInput").ap()
    wqk1 = nc.dram_tensor("wqk1", [P, 2, 4, P], BF16,
                          kind="ExternalInput").ap()
    wqk23 = nc.dram_tensor("wqk23", [P, 2, 4, 2 * P], BF16,
                           kind="ExternalInput").ap()
    wv = nc.dram_tensor("wv", [P, 4, E], BF16, kind="ExternalInput").ap()
    wo = nc.dram_tensor("wo", [P, 4, E], BF16, kind="ExternalInput").ap()
    # packed constants: [mask0(256) | mask1(256) | ident01(128) | bq(8) bk(8)]
    # biases are f32 pairs stored as raw bf16 slots (bitcast on device)
    CW = NT * SPAN + P + 16
    cst = nc.dram_tensor("cst", [P, CW], BF16, kind="ExternalInput").ap()
    yT = nc.dram_tensor("yT", [P, 4, SLOC], BF16, kind="ExternalOutput").ap()

    with tile.TileContext(nc) as tc:
        with (
            tc.tile_pool(name="consts", bufs=1) as consts,
            tc.tile_pool(name="persist", bufs=1) as persist,
            tc.tile_pool(name="work", bufs=4) as work,
            tc.tile_pool(name="ps_qkv", bufs=2, space="PSUM") as ps_qkv,
            tc.tile_pool(name="ps_sc", bufs=2, space="PSUM") as ps_sc,
            tc.tile_pool(name="ps_pt", bufs=1, space="PSUM") as ps_pt,
            tc.tile_pool(name="ps_av", bufs=1, space="PSUM") as ps_av,
            tc.tile_pool(name="ps_y", bufs=1, space="PSUM") as ps_y,
        ):
            def emit():
                # warm the PE clock gate during the load ramp: dummy
                # matmuls on a zeroed scratch tile, no data dependencies
                if WARMUP_MMS:
                    wsc = work.tile([P, E], BF16, tag="warm", name="warm")
                    nc.gpsimd.memset(wsc[:].bitcast(F32), 0.0)
                    # dummy activation: pulls the hoisted LoadActFuncSet to
                    # the head of the Act queue, off the critical path
                    wact = work.tile([P, 8], BF16, tag="wact", name="wact")
                    nc.scalar.activation(wact[:], wsc[:, 0:8], act_f.Exp)
                    wps = ps_qkv.tile([P, E], F32, tag="qkv", name="qkv")
                    for _ in range(WARMUP_MMS):
                        nc.tensor.matmul(wps[:], wsc[:, 0:P], wsc[:],
                                         start=True, stop=True)

                # ---- loads, in consumption order, spread across queues ----
                x_sb = persist.tile([P, 4, HALO], BF16, tag="x", name="x")
                wqk0_sb = persist.tile([P, 2, 4, P], BF16, tag="wqk0",
                                       name="wqk0_sb")
                wqk1_sb = persist.tile([P, 2, 4, P], BF16, tag="wqk1",
                                       name="wqk1_sb")
                wqk23_sb = persist.tile([P, 2, 4, 2 * P], BF16, tag="wqk23",
                                        name="wqk23_sb")
                wv_sb = persist.tile([P, 4, E], BF16, tag="wv", name="wv_sb")
                wo_sb = persist.tile([P, 4, E], BF16, tag="wo", name="wo_sb")
                cst_sb = consts.tile([P, CW], BF16, tag="cst", name="cst")

                nc.sync.dma_start(wqk0_sb[:], wqk0)
                nc.sync.dma_start(x_sb[:, 0:2, :], xT[:, 0:2, :])
                nc.sync.dma_start(x_sb[:, 2:4, :], xT[:, 2:4, :])
                nc.sync.dma_start(cst_sb[:], cst)
                nc.sync.dma_start(wqk1_sb[:], wqk1)
                nc.sync.dma_start(wv_sb[:], wv)
                nc.sync.dma_start(wqk23_sb[:], wqk23)
                nc.sync.dma_start(wo_sb[:], wo)

                mask_sb = [cst_sb[:, t * SPAN:(t + 1) * SPAN]
                           for t in range(NT)]
                ident01 = cst_sb[:, NT * SPAN:NT * SPAN + P]
                bias_f32 = cst_sb[:, NT * SPAN + P:].bitcast(F32)
                bq_sb = bias_f32[:, 0:4]
                bk_sb = bias_f32[:, 4:8]

                def wq_stat(c, k):
                    # stationary [128, 128] slice of wq for col-chunk c
                    if c == 0:
                        return wqk0_sb[:, 0, k, :]
                    if c == 1:
                        return wqk1_sb[:, 0, k, :]
                    return wqk23_sb[:, 0, k, (c - 2) * P:(c - 1) * P]

                def wk_stat(c, k):
                    if c == 0:
                        return wqk0_sb[:, 1, k, :]
                    if c == 1:
                        return wqk1_sb[:, 1, k, :]
                    return wqk23_sb[:, 1, k, (c - 2) * P:(c - 1) * P]

                # ---- QKV projections (emitted per-chunk, interleaved
                # with attention below) ----
                qT_sb, kT_sb = [None] * 4, [None] * 4

                def emit_q_chunk(c):
                    # q: [128, SLOC] from x cols [64, 64+256)
                    ps_q = ps_qkv.tile([P, E], F32, tag="qkv", name="qkv")
                    for k in range(4):
                        nc.tensor.matmul(
                            ps_q[:, 0:SLOC], wq_stat(c, k),
                            x_sb[:, k, HWIN:HWIN + SLOC],
                            start=(k == 0), stop=(k == 3),
                        )
                    sb_q = persist.tile([P, SLOC], BF16, tag=f"qT{c}",
                                        name=f"qT{c}")
                    nc.scalar.activation(sb_q[:], ps_q[:, 0:SLOC],
                                         act_f.Identity,
                                         bias=bq_sb[:, c:c + 1])
                    qT_sb[c] = sb_q

                def emit_k_chunk(c):
                    # k: [128, HALO]
                    ps_k = ps_qkv.tile([P, E], F32, tag="qkv", name="qkv")
                    for k in range(4):
                        nc.tensor.matmul(
                            ps_k[:, 0:HALO], wk_stat(c, k), x_sb[:, k, :],
                            start=(k == 0), stop=(k == 3),
                        )
                    sb_k = persist.tile([P, HALO], BF16, tag=f"kT{c}",
                                        name=f"kT{c}")
                    nc.vector.tensor_scalar_add(
                        sb_k[:], ps_k[:, 0:HALO], bk_sb[:, c:c + 1])
                    kT_sb[c] = sb_k

                def emit_qk_chunk(c):
                    emit_q_chunk(c)
                    emit_k_chunk(c)

                v_sb = [None] * KC

                def emit_v_chunk(skc):
                    ps = ps_qkv.tile([P, E], F32, tag="qkv", name="qkv")
                    for k in range(4):
                        nc.tensor.matmul(
                            ps[:], x_sb[:, k, skc * P:(skc + 1) * P],
                            wv_sb[:, k, :],
                            start=(k == 0), stop=(k == 3),
                        )
                    sb = persist.tile([P, E], BF16, tag=f"v{skc}",
                                      name=f"v{skc}")
                    if skc == 1:
                        nc.scalar.copy(sb[:], ps[:])
                    else:
                        nc.vector.tensor_copy(sb[:], ps[:])
                    v_sb[skc] = sb

                # per-pair absolute-grid attention buffers:
                # [p, head-local (2), block (c,t)->2c+t (6), q (128)];
                # blocks 1 (c0,t1) and 4 (c2,t0) are never written or read.
                pairbuf = [
                    persist.tile([P, 2, 6, P], BF16, tag=f"attT{pp}",
                                 name=f"attT{pp}")
                    for pp in range(4)
                ]

                valsT_sb = [
                    persist.tile([P, SLOC], BF16, tag=f"valsT{c}",
                                 name=f"valsT{c}")
                    for c in range(4)
                ]
                # yT accumulators: two PSUM banks, each holds 2 o-chunks
                y_ps = [ps_y.tile([P, SLOC], F32, tag=f"y{i}",
                                  name=f"y{i}") for i in range(2)]

                y_odd = [None, None]

                def emit_y_accum(f):
                    # one pending accumulation group per PSUM bank: even
                    # o-chunks accumulate in the ps_y banks; odd o-chunks
                    # pre-accumulate f0-2 in donated sc-pool banks
                    # (emit_y_odd) and finish with f3 in the tail
                    for j in range(2):
                        nc.tensor.matmul(
                            y_ps[j],
                            wo_sb[:, f, 2 * j * P:(2 * j + 1) * P],
                            valsT_sb[f][:], start=(f == 0), stop=(f == 3),
                        )

                def emit_y_odd():
                    for j in range(2):
                        y_odd[j] = ps_sc.tile([P, 2 * SPAN], F32, tag="sc",
                                              name="sc")
                        for f in range(3):
                            nc.tensor.matmul(
                                y_odd[j][:, 0:SLOC],
                                wo_sb[:, f, (2 * j + 1) * P:(2 * j + 2) * P],
                                valsT_sb[f][:], start=(f == 0), stop=False,
                            )

                def emit_y_tail():
                    for j in range(2):
                        nc.tensor.matmul(
                            y_odd[j][:, 0:SLOC],
                            wo_sb[:, 3, (2 * j + 1) * P:(2 * j + 2) * P],
                            valsT_sb[3][:], start=False, stop=True,
                        )

                scale = 1.0 / float(np.sqrt(HD))

                def emit_pass_b(pp):
                    # AV for the pair's two heads, split per q-half so the
                    # unwritten grid corners are never read
                    av = ps_av.tile([HD, 2, SLOC], F32, tag="av", name="av")
                    for t in range(NT):
                        for hl in range(2):
                            h = 2 * pp + hl
                            for kc in range(2):
                                c = t + kc
                                nc.tensor.matmul(
                                    av[:, hl, t * P:(t + 1) * P],
                                    v_sb[c][:, h * HD:(h + 1) * HD],
                                    pairbuf[pp][:, hl, 2 * c + t, :],
                                    start=(kc == 0), stop=(kc == 1),
                                )
                    eng = nc.vector if pp % 2 == 0 else nc.scalar
                    cp = eng.tensor_copy if pp % 2 == 0 else eng.copy
                    cp(valsT_sb[pp][0:HD, :], av[:, 0, :])
                    cp(valsT_sb[pp][HD:P, :], av[:, 1, :])
                    emit_y_accum(pp)

                # PASS A per head pair: both heads' scores into one PSUM
                # bank -> one wide exp -> per-head mask+rowsum on DVE ->
                # reciprocal -> diag(recip) folded into the PE transposes ->
                # one strided pair-copy into the absolute-key grid.
                ann = {}

                def stage_a(pp, t):
                    sc = ps_sc.tile([P, 2 * SPAN], F32, tag="sc",
                                    name="sc")
                    for hl in range(2):
                        # the additive band mask starts the group (its
                        # operands are full-partition, which the group
                        # leader requires); q.k accumulates on top
                        nc.tensor.matmul(
                            sc[:, hl * SPAN:(hl + 1) * SPAN],
                            ident01, mask_sb[t],
                            start=True, stop=False,
                        )
                        nc.tensor.matmul(
                            sc[:, hl * SPAN:(hl + 1) * SPAN],
                            qT_sb[pp][hl * HD:(hl + 1) * HD,
                                      t * P:(t + 1) * P],
                            kT_sb[pp][hl * HD:(hl + 1) * HD,
                                      t * P:t * P + SPAN],
                            start=False, stop=True,
                        )
                    p = work.tile([P, 2 * SPAN], BF16, tag="p", name="p")
                    sums = work.tile([P, 2], F32, tag="sums", name="sums")
                    recip = work.tile([P, 2], F32, tag="recip",
                                      name="recip")
                    nc.scalar.activation(p[:], sc[:], act_f.Exp,
                                         scale=scale)
                    psum_d = work.tile([P, 2 * SPAN], BF16, tag="psd",
                                       name="psd")
                    for hl in range(2):
                        nc.vector.tensor_scalar(
                            psum_d[:, hl * SPAN:(hl + 1) * SPAN],
                            p[:, hl * SPAN:(hl + 1) * SPAN], 1.0, 0.0,
                            op0=mybir.AluOpType.mult,
                            op1=mybir.AluOpType.add,
                            accum_out=sums[:, hl:hl + 1])
                    nc.vector.reciprocal(recip[:], sums[:])
                    an = work.tile([P, 2 * SPAN], BF16, tag="an",
                                   name="an")
                    for hl in range(2):
                        eng = nc.vector if hl == 0 else nc.gpsimd
                        eng.tensor_scalar_mul(
                            an[:, hl * SPAN:(hl + 1) * SPAN],
                            p[:, hl * SPAN:(hl + 1) * SPAN],
                            recip[:, hl:hl + 1])
                    ann[(pp, t)] = an

                def stage_b(pp, t):
                    an = ann[(pp, t)]
                    ptp = ps_pt.tile([P, 2, 2, P], BF16, tag="pt",
                                     name="pt")
                    for hl in range(2):
                        for kc in range(2):
                            nc.tensor.transpose(
                                ptp[:, hl, kc, :],
                                an[:, hl * SPAN + kc * P:
                                   hl * SPAN + (kc + 1) * P],
                                ident01,
                            )
                    # blocks (c=t+kc, t) -> index 2*(t+kc)+t = 3t+2kc
                    dst = pairbuf[pp][:, :, 3 * t:3 * t + 3:2, :]
                    nc.vector.tensor_copy(dst, ptp[:])

                # software-pipelined schedule: pair pp's transposes run one
                # iteration after its scores/exp, so the next pair's scores
                # sit ahead of them in the in-order PE queue
                ops = {
                    "qk": emit_qk_chunk, "sa": stage_a, "sb": stage_b,
                    "qq": emit_q_chunk, "kk": emit_k_chunk,
                    "v": emit_v_chunk, "pb": emit_pass_b,
                    "yo": lambda: emit_y_odd(),
                }
                for step in SCHEDULE:
                    ops[step[0]](*step[1:])

                emit_pass_b(3)
                emit_y_tail()

                # ---- output: evacuate y PSUM and store ----
                ysb = work.tile([P, 4, SLOC], BF16, tag="yt", name="yt")
                ysb3 = ysb[:]
                nc.scalar.copy(ysb3[:, 0, :], y_ps[0])
                nc.vector.tensor_copy(ysb3[:, 1, :], y_odd[0][:, 0:SLOC])
                nc.scalar.copy(ysb3[:, 2, :], y_ps[1])
                nc.vector.tensor_copy(ysb3[:, 3, :], y_odd[1][:, 0:SLOC])
                nc.sync.dma_start(yT[:, 0:2, :], ysb3[:, 0:2, :])
                nc.sync.dma_start(yT[:, 2:4, :], ysb3[:, 2:4, :])

            for _rep in range(reps):
                emit()

    return nc


_prog_cache = {}


def _get_program(reps: int = 1):
    key = (WARMUP_MMS, reps, CHAIN_MODE, tuple(map(tuple, SCHEDULE)))
    if key not in _prog_cache:
        nc = bacc.Bacc(
            "TRN2", target_bir_lowering=False, debug=False,
            num_devices=N_CORES,
        )
        _build_kernel(nc, reps=reps)
        nc.compile()
        _prog_cache[key] = nc
    return _prog_cache[key]


def _bf16(a):
    import ml_dtypes
    return np.ascontiguousarray(a.astype(ml_dtypes.bfloat16))


def _make_in_maps(x, w_qkv, b_qkv, w_o):
    import ml_dtypes
    x2 = np.ascontiguousarray(np.asarray(x, np.float32).reshape(S, E))
    w_qkv = np.asarray(w_qkv, np.float32)
    b_qkv = np.asarray(b_qkv, np.float32)
    w_o = np.asarray(w_o, np.float32)

    # w_qkv rows for head h: [h*3hd, h*3hd+hd) = q, +hd = k, +2hd = v
    idx_q = np.concatenate(
        [np.arange(h * 3 * HD, h * 3 * HD + HD) for h in range(H)])
    idx_k = idx_q + HD
    idx_v = idx_q + 2 * HD
    wqT = w_qkv[idx_q].T   # [in, (h,d)]
    wkT = w_qkv[idx_k].T
    wvT = w_qkv[idx_v].T
    woT = w_o.T            # [(h,d), out]

    # [p, q/k, kchunk, cols]: stat slice (c,k) = wT[k*128+p, c*128+col]
    def qk_cols(c0, c1):
        n = c1 - c0
        out = np.empty((P, 2, 4, n), np.float32)
        for k in range(4):
            out[:, 0, k, :] = wqT[k * P:(k + 1) * P, c0:c1]
            out[:, 1, k, :] = wkT[k * P:(k + 1) * P, c0:c1]
        return _bf16(out)

    wqk0_h = qk_cols(0, P)
    wqk1_h = qk_cols(P, 2 * P)
    wqk23_h = qk_cols(2 * P, 4 * P)
    wv_h = _bf16(wvT.reshape(4, P, E).transpose(1, 0, 2))
    wo_h = _bf16(woT.reshape(4, P, E).transpose(1, 0, 2))

    bq = b_qkv[idx_q].reshape(4, P).T  # [p, chunk] f32
    bk = b_qkv[idx_k].reshape(4, P).T
    bias = np.ascontiguousarray(
        np.concatenate([bq, bk], axis=1).astype(np.float32))    # [p, 8]
    bias_bf = bias.view(np.uint16).view(ml_dtypes.bfloat16)     # [p, 16]
    ident = np.eye(P, dtype=np.float32)

    xT = x2.T  # [E, S]
    in_maps = []
    for core in range(N_CORES):
        q0 = core * SLOC
        lo = q0 - HWIN
        xt = np.zeros((E, HALO), np.float32)
        slo, shi = max(lo, 0), min(q0 + SLOC + HWIN, S)
        xt[:, slo - lo: shi - lo] = xT[:, slo:shi]
        xt_h = _bf16(xt.reshape(4, P, HALO).transpose(1, 0, 2))

        m = np.zeros((NT, P, SPAN), np.float32)
        for t in range(NT):
            # key position for span col j: q0 + t*128 - 64 + j
            kpos = q0 + t * P - HWIN + np.arange(SPAN)
            qpos = (q0 + t * P + np.arange(P))[:, None]
            valid = (np.abs(kpos[None, :] - qpos) <= HWIN) \
                & (kpos[None, :] >= 0) & (kpos[None, :] < S)
            m[t] = np.where(valid, 0.0, -1e30).astype(np.float32)

        cst = np.concatenate(
            [_bf16(m[0]), _bf16(m[1]), _bf16(ident),
             np.asarray(bias_bf)], axis=1)
        in_maps.append({
            "xT": np.ascontiguousarray(xt_h),
            "wqk0": wqk0_h, "wqk1": wqk1_h, "wqk23": wqk23_h,
            "wv": wv_h, "wo": wo_h,
            "cst": np.ascontiguousarray(cst),
        })
    return in_maps


last_result = None  # BassKernelResults of the most recent run (for profiling)


def kernel(x, padding_mask, w_qkv, b_qkv, w_o, b_o, trace=False):
    global last_result
    b_qkv = np.asarray(b_qkv, np.float32)
    w_o = np.asarray(w_o, np.float32)
    b_o = np.asarray(b_o, np.float32)
    idx_v = np.concatenate(
        [np.arange(h * 3 * HD + 2 * HD, (h + 1) * 3 * HD) for h in range(H)])
    # rows of softmax sum to 1 (padding_mask is all ones per spec), so the
    # v/out biases commute to a constant output offset; b_qkv[idx_v] is in
    # (h,d) order, matching w_o's input order
    b_eff = b_o + w_o @ b_qkv[idx_v]

    nc = _get_program()
    in_maps = _make_in_maps(x, w_qkv, b_qkv, w_o)
    res = run_bass_kernel_spmd(
        nc, in_maps, core_ids=list(range(N_CORES)), trace=trace)
    last_result = res
    # yT [p, o, q] bf16 per core; row e = o*128+p -> y block [256, 512]
    y = np.concatenate(
        [np.asarray(r["yT"], np.float32).transpose(1, 0, 2)
         .reshape(4 * P, SLOC).T
         for r in res.results], axis=0)  # [S, E]
    y = y + b_eff[None, :]
    return y.reshape(1, S, E).astype(np.float32)
